# revision 15
# baseline (speedup 1.0000x reference)
"""CPSF fused codebook kernel for Trainium2 (8 NeuronCores, codebook-parallel).

Sharding: M (codebook, 4096) split 8 ways -> 512 entries/core; every core sees
all B=2048 queries (large free dim amortizes per-instruction overhead). Host
sums the 8 partial [B,S] outputs.

Per (b,m,k):  Phi_k = ln(alpha w_k) + G*q_par_k + c_o*q_perp + c_o*dist_d
              wgt = sum_k exp(Phi_k);  T = wgt @ That
Factored:     base = sgn*|Gd|*(x^2+y^2) + F3   (F3: one PE-accumulated field,
              holds all q0/dist_d/cross/log terms + the umid*x range shift)
              E_k = exp(u'_k[m]*x + v'_k[m])   (ACT per-partition scale/bias)
              wgt = exp(base) * sum_k E_k
"""

import numpy as np

B, M, N, S, K = 2048, 4096, 64, 64, 8
EPS = 1e-3
NCORES = 8
ML = M // NCORES          # 512 codebook entries per core
MT = ML // 128            # 4 m-tiles per core
NQ = 4                    # b-quarters (PSUM-sized chunks of 512)
BQ = B // NQ              # 512
f32 = np.float32

_CACHE = {}


def _prep(z_re, z_im, d_re, d_im, zj_re, zj_im, dj_re, dj_im,
          That_re, That_im, alpha, sig_par, sig_perp):
    """Host-side packing: fp64 exact, cast to fp32 at the end."""
    x64 = lambda a: np.asarray(a, np.float64)
    zr, zi, dr, di = map(x64, (z_re, z_im, d_re, d_im))
    zjr, zji, djr, dji = map(x64, (zj_re, zj_im, dj_re, dj_im))

    tgl, wgl = np.polynomial.legendre.leggauss(K)
    t = (0.5 * (tgl + 1.0)).astype(f32).astype(np.float64)
    wq = (0.5 * wgl).astype(f32).astype(np.float64)

    dd2 = (djr**2 + dji**2).sum(-1)                          # [M]
    c_re = (djr * zjr + dji * zji).sum(-1)
    c_im = (djr * zji - dji * zjr).sum(-1)
    sp2 = x64(sig_par)**2 + EPS
    so2 = x64(sig_perp)**2 + EPS
    G = -0.5 / sp2
    c_o = -0.5 / so2
    Gd = G - c_o
    umid = -G * dd2
    lnal = np.log(np.maximum(x64(alpha), 1e-38))
    nzj = (zjr**2 + zji**2).sum(-1)
    nz = (zr**2 + zi**2).sum(-1)                             # [B]
    nd = (dr**2 + di**2).sum(-1)

    u = np.stack([-2.0 * G * t[k] * dd2 for k in range(K)])  # [K,M]
    up = u - umid[None, :]
    vp = np.stack([np.log(wq[k]) + G * (t[k] * dd2)**2 - up[k] * c_re
                   for k in range(K)])

    djx = np.concatenate([djr.T, dji.T], 0)                  # [128, M]
    djy = np.concatenate([-dji.T, djr.T], 0)
    f3z = ((-2.0 * c_o) * np.concatenate([zjr.T, zji.T], 0)
           + (-2.0 * Gd * c_re + umid) * djx
           + (-2.0 * Gd * c_im) * djy)
    f3d = (-2.0 * c_o) * np.concatenate([djr.T, dji.T], 0)
    const0 = (c_o * (nzj + dd2) + Gd * (c_re**2 + c_im**2)
              + lnal - umid * c_re)
    f3c = np.stack([const0, c_o, c_o])                       # [3, M]
    rhsc = np.stack([np.ones(B), nz, nd])                    # [3, B]
    that2 = np.concatenate([x64(That_re), x64(That_im)], 1)  # [M, 128]

    # pk: per m-tile block [djx | djy | f3z | f3d], each 128 cols
    nt = M // 128
    pk = np.empty((128, nt * 512), np.float64)
    for j in range(nt):
        ms = slice(j * 128, (j + 1) * 128)
        pk[:, j * 512 + 0:j * 512 + 128] = djx[:, ms]
        pk[:, j * 512 + 128:j * 512 + 256] = djy[:, ms]
        pk[:, j * 512 + 256:j * 512 + 384] = f3z[:, ms]
        pk[:, j * 512 + 384:j * 512 + 512] = f3d[:, ms]

    psq = np.sqrt(np.abs(Gd)).reshape(nt, 128).T             # [128, nt]
    psgn = np.where(Gd >= 0, 1.0, -1.0).reshape(nt, 128).T
    pu = up.reshape(K, nt, 128).transpose(2, 0, 1).reshape(128, K * nt)
    pv = vp.reshape(K, nt, 128).transpose(2, 0, 1).reshape(128, K * nt)

    c = lambda a: np.ascontiguousarray(a, dtype=f32)
    return dict(pk=c(pk), f3c=c(f3c), that2=c(that2), psq=c(psq),
                psgn=c(psgn), pu=c(pu), pv=c(pv),
                zst=c(zst_ := np.concatenate([zr.T, zi.T], 0)),
                dst=c(np.concatenate([dr.T, di.T], 0)), rhsc=c(rhsc))


def _core_slices(p, cid):
    """Per-core in_map from the full packed arrays (m-sharded)."""
    jt = slice(cid * MT * 512, (cid + 1) * MT * 512)         # pk cols
    ms = slice(cid * ML, (cid + 1) * ML)
    jc = slice(cid * MT, (cid + 1) * MT)
    kc = np.concatenate([np.arange(k * (M // 128) + cid * MT,
                                   k * (M // 128) + (cid + 1) * MT)
                         for k in range(K)])
    cc = np.ascontiguousarray
    return {"pk": cc(p["pk"][:, jt]), "f3c": cc(p["f3c"][:, ms]),
            "that2": cc(p["that2"][ms, :]), "psq": cc(p["psq"][:, jc]),
            "psgn": cc(p["psgn"][:, jc]), "pu": cc(p["pu"][:, kc]),
            "pv": cc(p["pv"][:, kc]), "zst": p["zst"], "dst": p["dst"],
            "rhsc": p["rhsc"]}


def _device_maps(maps):
    dev_maps = []
    for m in maps:
        dm = {k: m[k] for k in ("pk", "f3c", "that2", "zst", "dst", "rhsc")}
        dm["params"] = np.ascontiguousarray(np.concatenate(
            [m["psq"], m["psgn"], m["pu"], m["pv"]], axis=1))
        dev_maps.append(dm)
    return dev_maps


def _emulate_core(m):
    """Numpy emulation of one core's device program."""
    zst, dst, rhsc = m["zst"], m["dst"], m["rhsc"]
    t_acc = np.zeros((128, 2048), f32)
    for j in range(MT):
        pkj = m["pk"][:, j * 512:(j + 1) * 512]
        djx_t, djy_t = pkj[:, 0:128], pkj[:, 128:256]
        f3z_t, f3d_t = pkj[:, 256:384], pkj[:, 384:512]
        x = (djx_t.T @ zst).astype(f32)
        y = (djy_t.T @ zst).astype(f32)
        F3 = (f3z_t.T @ zst + f3d_t.T @ dst
              + m["f3c"][:, j * 128:(j + 1) * 128].T @ rhsc).astype(f32)
        sq = m["psq"][:, j:j + 1]
        xx = np.square(x * sq, dtype=f32)
        yy = np.square(y * sq, dtype=f32)
        Q = (xx + yy).astype(f32)
        Qs = (Q * m["psgn"][:, j:j + 1]).astype(f32)
        base = (F3 + Qs).astype(f32)
        Ssum = np.zeros_like(x)
        for k in range(K):
            col = k * MT + j
            arg = (x * m["pu"][:, col:col + 1] + m["pv"][:, col:col + 1]).astype(f32)
            Ssum = (Ssum + np.exp(arg, dtype=f32)).astype(f32)
        wgt = (np.exp(base, dtype=f32) * Ssum).astype(f32)
        that_t = m["that2"][j * 128:(j + 1) * 128, :]
        t_acc += (that_t.T @ wgt).astype(f32)
    return t_acc


def _build_bass():
    import concourse.bacc as bacc
    import concourse.mybir as mybir
    from concourse import tile

    dt = mybir.dt.float32
    AF = mybir.ActivationFunctionType
    AO = mybir.AluOpType
    nc = bacc.Bacc("TRN2", target_bir_lowering=False, debug=False)

    dram = {}
    for name, shape in [("zst", [128, B]), ("dst", [128, B]),
                        ("rhsc", [3, B]), ("pk", [128, MT * 512]),
                        ("f3c", [3, ML]), ("that2", [ML, 128]),
                        ("params", [128, 2 * MT + 2 * K * MT])]:
        dram[name] = nc.dram_tensor(name, shape, dt, kind="ExternalInput")
    tout = nc.dram_tensor("tout", [128, B], dt, kind="ExternalOutput")

    with tile.TileContext(nc) as tc:
        with tc.tile_pool(name="const", bufs=1) as cpool:
            zst = cpool.tile([128, B], dt)
            dst = cpool.tile([128, B], dt)
            rhsc = cpool.tile([3, B], dt)
            params = cpool.tile([128, 2 * MT + 2 * K * MT], dt)
            psq = params[:, 0:MT]
            psgn = params[:, MT:2 * MT]
            pu = params[:, 2 * MT:2 * MT + K * MT]
            pv = params[:, 2 * MT + K * MT:2 * MT + 2 * K * MT]
            that_all = cpool.tile([128, MT * 128], dt)
            f3c_all = cpool.tile([3, ML], dt)
            for t_, d_ in [(zst, "zst"), (dst, "dst"), (rhsc, "rhsc"),
                           (params, "params"), (f3c_all, "f3c")]:
                nc.sync.dma_start(t_[:, :], dram[d_][:, :])
            nc.sync.dma_start(
                that_all[:, :].rearrange("p (j c) -> p j c", j=MT),
                dram["that2"][:, :].rearrange("(j p) c -> p j c", p=128))

            with (
                tc.tile_pool(name="lhs", bufs=2) as lpool,
                tc.tile_pool(name="work", bufs=1) as wpool,
                tc.tile_pool(name="eslab", bufs=1) as epool,
                tc.tile_pool(name="wgtp", bufs=2) as gpool,
                tc.tile_pool(name="xps", bufs=1, space="PSUM") as xpool,
                tc.tile_pool(name="fq", bufs=1, space="PSUM") as qpool,
                tc.tile_pool(name="tps", bufs=1, space="PSUM") as tpool,
            ):
                HB = B // 2                      # 1024: b-half for x/E passes
                tp = tpool.tile([128, B], dt, tag="tp")   # T^T accumulator
                for j in range(MT):
                    pk_t = lpool.tile([128, 512], dt, tag="pk")
                    nc.sync.dma_start(pk_t[:, :],
                                      dram["pk"][:, j * 512:(j + 1) * 512])
                    djx_t = pk_t[:, 0:128]
                    djy_t = pk_t[:, 128:256]
                    f3z_t = pk_t[:, 256:384]
                    f3d_t = pk_t[:, 384:512]
                    f3c_t = f3c_all[:, j * 128:(j + 1) * 128]

                    xx = wpool.tile([128, B], dt, tag="xx")
                    yy = wpool.tile([128, B], dt, tag="yy")
                    EB = wpool.tile([128, B], dt, tag="EB")
                    slabs = [epool.tile([128, 2 * B], dt, tag=f"esl{p}",
                                        name=f"esl{p}_{j}")
                             for p in range(4)]
                    wgt = gpool.tile([128, B], dt, tag="wgt")

                    for h in range(2):
                        hs = slice(h * HB, (h + 1) * HB)
                        x_h = xpool.tile([128, HB], dt, tag="x")
                        for q2 in range(2):
                            qg = h * 2 + q2
                            qs = slice(qg * BQ, (qg + 1) * BQ)
                            nc.tensor.matmul(x_h[:, q2 * BQ:(q2 + 1) * BQ],
                                             djx_t, zst[:, qs],
                                             start=True, stop=True)
                        for k in range(K):
                            col = k * MT + j
                            nc.scalar.activation(
                                slabs[k // 2][:, (k % 2) * B + h * HB:
                                              (k % 2) * B + (h + 1) * HB],
                                x_h[:, :], AF.Exp,
                                bias=pv[:, col:col + 1],
                                scale=pu[:, col:col + 1])
                        nc.scalar.activation(xx[:, hs], x_h[:, :], AF.Square,
                                             scale=psq[:, j:j + 1])
                        # y / F3 quarters for this half; base assembled
                        # quarter-wise (in-place into xx) to free PSUM early
                        for q2 in range(2):
                            qg = h * 2 + q2
                            qs = slice(qg * BQ, (qg + 1) * BQ)
                            yq = qpool.tile([128, BQ], dt, tag="yq")
                            nc.tensor.matmul(yq[:, :], djy_t, zst[:, qs],
                                             start=True, stop=True)
                            nc.scalar.activation(yy[:, qs], yq[:, :],
                                                 AF.Square,
                                                 scale=psq[:, j:j + 1])
                            f3q = qpool.tile([128, BQ], dt, tag="f3q")
                            nc.tensor.matmul(f3q[:, :], f3z_t, zst[:, qs],
                                             start=True, stop=False)
                            nc.tensor.matmul(f3q[:, :], f3d_t, dst[:, qs],
                                             start=False, stop=False)
                            nc.tensor.matmul(f3q[:, :], f3c_t, rhsc[:, qs],
                                             start=False, stop=True)
                            nc.vector.tensor_add(xx[:, qs], xx[:, qs],
                                                 yy[:, qs])
                            nc.vector.tensor_scalar(xx[:, qs], xx[:, qs],
                                                    psgn[:, j:j + 1], None,
                                                    AO.mult)
                            nc.vector.tensor_add(xx[:, qs], xx[:, qs],
                                                 f3q[:, :])

                    nc.scalar.activation(EB[:, :], xx[:, :], AF.Exp)

                    sa = wpool.tile([128, B], dt, tag="sa")
                    sb = wpool.tile([128, B], dt, tag="sb")
                    sc = wpool.tile([128, B], dt, tag="sc")
                    nc.vector.tensor_add(sa[:, :], slabs[0][:, 0:B],
                                         slabs[0][:, B:2 * B])
                    nc.vector.tensor_add(sb[:, :], slabs[1][:, 0:B],
                                         slabs[1][:, B:2 * B])
                    nc.vector.tensor_add(sa[:, :], sa[:, :], sb[:, :])
                    nc.vector.tensor_add(sb[:, :], slabs[2][:, 0:B],
                                         slabs[2][:, B:2 * B])
                    nc.vector.tensor_add(sc[:, :], slabs[3][:, 0:B],
                                         slabs[3][:, B:2 * B])
                    nc.vector.tensor_add(sb[:, :], sb[:, :], sc[:, :])
                    nc.vector.tensor_add(sa[:, :], sa[:, :], sb[:, :])
                    nc.vector.tensor_mul(wgt[:, :], EB[:, :], sa[:, :])

                    # T^T += That_j.T @ wgt  (That stationary, wgt moving)
                    that_j = that_all[:, j * 128:(j + 1) * 128]
                    for q in range(NQ):
                        qs = slice(q * BQ, (q + 1) * BQ)
                        nc.tensor.matmul(tp[:, qs], that_j, wgt[:, qs],
                                         start=(j == 0), stop=(j == MT - 1))

                ocp = wpool.tile([128, B], dt, tag="xx")
                nc.vector.tensor_copy(ocp[:, :], tp[:, :])
                nc.sync.dma_start(tout[:, :], ocp[:, :])

    nc.compile()
    return nc


def kernel(z_re, z_im, d_re, d_im, zj_re, zj_im, dj_re, dj_im,
           That_re, That_im, alpha, sig_par, sig_perp, _emulate=False):
    p = _prep(z_re, z_im, d_re, d_im, zj_re, zj_im, dj_re, dj_im,
              That_re, That_im, alpha, sig_par, sig_perp)
    maps = [_core_slices(p, c) for c in range(NCORES)]

    if _emulate:
        outs = [_emulate_core(m) for m in maps]
    else:
        from concourse.bass_utils import run_bass_kernel_spmd
        if "nc" not in _CACHE:
            _CACHE["nc"] = _build_bass()
        dev_maps = _device_maps(maps)
        res = run_bass_kernel_spmd(_CACHE["nc"], dev_maps,
                                   core_ids=list(range(NCORES)))
        outs = [res.results[c]["tout"] for c in range(NCORES)]

    full = np.zeros((128, B), np.float64)
    for o in outs:
        full += o.astype(np.float64)
    full = full.astype(f32).T                   # [B, 128]
    return (full[:, :S] + 1j * full[:, S:]).astype(np.complex64)


# revision 16
# speedup vs baseline: 1.2801x; 1.2801x over previous
"""CPSF fused codebook kernel for Trainium2 (8 NeuronCores, codebook-parallel).

Sharding: M (codebook, 4096) split 8 ways -> 512 entries/core; every core sees
all B=2048 queries (large free dim amortizes per-instruction overhead). Host
sums the 8 partial [B,S] outputs.

Per (b,m,k):  Phi_k = ln(alpha w_k) + G*q_par_k + c_o*q_perp + c_o*dist_d
              wgt = sum_k exp(Phi_k);  T = wgt @ That
Factored:     base = sgn*|Gd|*(x^2+y^2) + F3   (F3: one PE-accumulated field,
              holds all q0/dist_d/cross/log terms + the umid*x range shift)
              E_k = exp(u'_k[m]*x + v'_k[m])   (ACT per-partition scale/bias)
              wgt = exp(base) * sum_k E_k
"""

import numpy as np

B, M, N, S, K = 2048, 4096, 64, 64, 8
EPS = 1e-3
NCORES = 8
ML = M // NCORES          # 512 codebook entries per core
MT = ML // 128            # 4 m-tiles per core
NQ = 4                    # b-quarters (PSUM-sized chunks of 512)
BQ = B // NQ              # 512
f32 = np.float32

_CACHE = {}


def _prep(z_re, z_im, d_re, d_im, zj_re, zj_im, dj_re, dj_im,
          That_re, That_im, alpha, sig_par, sig_perp):
    """Host-side packing: fp64 exact, cast to fp32 at the end."""
    x64 = lambda a: np.asarray(a, np.float64)
    zr, zi, dr, di = map(x64, (z_re, z_im, d_re, d_im))
    zjr, zji, djr, dji = map(x64, (zj_re, zj_im, dj_re, dj_im))

    tgl, wgl = np.polynomial.legendre.leggauss(K)
    t = (0.5 * (tgl + 1.0)).astype(f32).astype(np.float64)
    wq = (0.5 * wgl).astype(f32).astype(np.float64)

    dd2 = (djr**2 + dji**2).sum(-1)                          # [M]
    c_re = (djr * zjr + dji * zji).sum(-1)
    c_im = (djr * zji - dji * zjr).sum(-1)
    sp2 = x64(sig_par)**2 + EPS
    so2 = x64(sig_perp)**2 + EPS
    G = -0.5 / sp2
    c_o = -0.5 / so2
    Gd = G - c_o
    umid = -G * dd2
    lnal = np.log(np.maximum(x64(alpha), 1e-38))
    nzj = (zjr**2 + zji**2).sum(-1)
    nz = (zr**2 + zi**2).sum(-1)                             # [B]
    nd = (dr**2 + di**2).sum(-1)

    u = np.stack([-2.0 * G * t[k] * dd2 for k in range(K)])  # [K,M]
    up = u - umid[None, :]
    vp = np.stack([np.log(wq[k]) + G * (t[k] * dd2)**2 - up[k] * c_re
                   for k in range(K)])

    djx = np.concatenate([djr.T, dji.T], 0)                  # [128, M]
    djy = np.concatenate([-dji.T, djr.T], 0)
    f3z = ((-2.0 * c_o) * np.concatenate([zjr.T, zji.T], 0)
           + (-2.0 * Gd * c_re + umid) * djx
           + (-2.0 * Gd * c_im) * djy)
    f3d = (-2.0 * c_o) * np.concatenate([djr.T, dji.T], 0)
    const0 = (c_o * (nzj + dd2) + Gd * (c_re**2 + c_im**2)
              + lnal - umid * c_re)
    f3c = np.stack([const0, c_o, c_o])                       # [3, M]
    rhsc = np.stack([np.ones(B), nz, nd])                    # [3, B]
    that2 = np.concatenate([x64(That_re), x64(That_im)], 1)  # [M, 128]

    # pk: per m-tile block [djx | djy | f3z | f3d], each 128 cols
    nt = M // 128
    pk = np.empty((128, nt * 512), np.float64)
    for j in range(nt):
        ms = slice(j * 128, (j + 1) * 128)
        pk[:, j * 512 + 0:j * 512 + 128] = djx[:, ms]
        pk[:, j * 512 + 128:j * 512 + 256] = djy[:, ms]
        pk[:, j * 512 + 256:j * 512 + 384] = f3z[:, ms]
        pk[:, j * 512 + 384:j * 512 + 512] = f3d[:, ms]

    psq = np.sqrt(np.abs(Gd)).reshape(nt, 128).T             # [128, nt]
    psgn = np.where(Gd >= 0, 1.0, -1.0).reshape(nt, 128).T
    pu = up.reshape(K, nt, 128).transpose(2, 0, 1).reshape(128, K * nt)
    pv = vp.reshape(K, nt, 128).transpose(2, 0, 1).reshape(128, K * nt)

    c = lambda a: np.ascontiguousarray(a, dtype=f32)
    return dict(pk=c(pk), f3c=c(f3c), that2=c(that2), psq=c(psq),
                psgn=c(psgn), pu=c(pu), pv=c(pv),
                zst=c(zst_ := np.concatenate([zr.T, zi.T], 0)),
                dst=c(np.concatenate([dr.T, di.T], 0)), rhsc=c(rhsc))


def _core_slices(p, cid):
    """Per-core in_map from the full packed arrays (m-sharded)."""
    jt = slice(cid * MT * 512, (cid + 1) * MT * 512)         # pk cols
    ms = slice(cid * ML, (cid + 1) * ML)
    jc = slice(cid * MT, (cid + 1) * MT)
    kc = np.concatenate([np.arange(k * (M // 128) + cid * MT,
                                   k * (M // 128) + (cid + 1) * MT)
                         for k in range(K)])
    cc = np.ascontiguousarray
    return {"pk": cc(p["pk"][:, jt]), "f3c": cc(p["f3c"][:, ms]),
            "that2": cc(p["that2"][ms, :]), "psq": cc(p["psq"][:, jc]),
            "psgn": cc(p["psgn"][:, jc]), "pu": cc(p["pu"][:, kc]),
            "pv": cc(p["pv"][:, kc]), "zst": p["zst"], "dst": p["dst"],
            "rhsc": p["rhsc"]}


def _device_maps(maps):
    dev_maps = []
    for m in maps:
        dm = {k: m[k] for k in ("pk", "f3c", "that2", "zst", "dst", "rhsc")}
        dm["params"] = np.ascontiguousarray(np.concatenate(
            [m["psq"], m["psgn"], m["pu"], m["pv"]], axis=1))
        dev_maps.append(dm)
    return dev_maps


def _emulate_core(m):
    """Numpy emulation of one core's device program."""
    zst, dst, rhsc = m["zst"], m["dst"], m["rhsc"]
    t_acc = np.zeros((128, 2048), f32)
    for j in range(MT):
        pkj = m["pk"][:, j * 512:(j + 1) * 512]
        djx_t, djy_t = pkj[:, 0:128], pkj[:, 128:256]
        f3z_t, f3d_t = pkj[:, 256:384], pkj[:, 384:512]
        x = (djx_t.T @ zst).astype(f32)
        y = (djy_t.T @ zst).astype(f32)
        F3 = (f3z_t.T @ zst + f3d_t.T @ dst
              + m["f3c"][:, j * 128:(j + 1) * 128].T @ rhsc).astype(f32)
        sq = m["psq"][:, j:j + 1]
        xx = np.square(x * sq, dtype=f32)
        yy = np.square(y * sq, dtype=f32)
        Q = (xx + yy).astype(f32)
        Qs = (Q * m["psgn"][:, j:j + 1]).astype(f32)
        base = (F3 + Qs).astype(f32)
        Ssum = np.zeros_like(x)
        for k in range(K):
            col = k * MT + j
            arg = (x * m["pu"][:, col:col + 1] + m["pv"][:, col:col + 1]).astype(f32)
            Ssum = (Ssum + np.exp(arg, dtype=f32)).astype(f32)
        wgt = (np.exp(base, dtype=f32) * Ssum).astype(f32)
        that_t = m["that2"][j * 128:(j + 1) * 128, :]
        t_acc += (that_t.T @ wgt).astype(f32)
    return t_acc


def _build_bass():
    import concourse.bacc as bacc
    import concourse.mybir as mybir
    from concourse import tile

    dt = mybir.dt.float32
    AF = mybir.ActivationFunctionType
    AO = mybir.AluOpType
    nc = bacc.Bacc("TRN2", target_bir_lowering=False, debug=False)

    dram = {}
    for name, shape in [("zst", [128, B]), ("dst", [128, B]),
                        ("rhsc", [3, B]), ("pk", [128, MT * 512]),
                        ("f3c", [3, ML]), ("that2", [ML, 128]),
                        ("params", [128, 2 * MT + 2 * K * MT])]:
        dram[name] = nc.dram_tensor(name, shape, dt, kind="ExternalInput")
    tout = nc.dram_tensor("tout", [128, B], dt, kind="ExternalOutput")

    with tile.TileContext(nc) as tc:
        with tc.tile_pool(name="const", bufs=1) as cpool:
            zst = cpool.tile([128, B], dt)
            dst = cpool.tile([128, B], dt)
            rhsc = cpool.tile([3, B], dt)
            params = cpool.tile([128, 2 * MT + 2 * K * MT], dt)
            psq = params[:, 0:MT]
            psgn = params[:, MT:2 * MT]
            pu = params[:, 2 * MT:2 * MT + K * MT]
            pv = params[:, 2 * MT + K * MT:2 * MT + 2 * K * MT]
            that_all = cpool.tile([128, MT * 128], dt)
            f3c_all = cpool.tile([3, ML], dt)
            for t_, d_ in [(zst, "zst"), (dst, "dst"), (rhsc, "rhsc"),
                           (params, "params"), (f3c_all, "f3c")]:
                nc.sync.dma_start(t_[:, :], dram[d_][:, :])
            nc.sync.dma_start(
                that_all[:, :].rearrange("p (j c) -> p j c", j=MT),
                dram["that2"][:, :].rearrange("(j p) c -> p j c", p=128))

            with (
                tc.tile_pool(name="lhs", bufs=2) as lpool,
                tc.tile_pool(name="work", bufs=1) as wpool,
                tc.tile_pool(name="eslab", bufs=1) as epool,
                tc.tile_pool(name="wgtp", bufs=1) as gpool,
            ):
                wgts = []
                HB = B // 2                      # 1024: b-half for x/E passes
                with (
                    tc.tile_pool(name="xps", bufs=2, space="PSUM") as xpool,
                    tc.tile_pool(name="fq", bufs=2, space="PSUM") as qpool,
                ):
                    for j in range(MT):
                        pk_t = lpool.tile([128, 512], dt, tag="pk")
                        nc.sync.dma_start(pk_t[:, :],
                                          dram["pk"][:, j * 512:(j + 1) * 512])
                        djx_t = pk_t[:, 0:128]
                        djy_t = pk_t[:, 128:256]
                        f3z_t = pk_t[:, 256:384]
                        f3d_t = pk_t[:, 384:512]
                        f3c_t = f3c_all[:, j * 128:(j + 1) * 128]

                        xx = wpool.tile([128, B], dt, tag="xx")
                        yy = wpool.tile([128, B], dt, tag="yy")
                        EB = wpool.tile([128, B], dt, tag="EB")
                        slabs = [epool.tile([128, 2 * B], dt, tag=f"esl{p}",
                                            name=f"esl{p}_{j}")
                                 for p in range(4)]
                        wgt = gpool.tile([128, B], dt, tag=f"wgt{j}")

                        for h in range(2):
                            hs = slice(h * HB, (h + 1) * HB)
                            x_h = xpool.tile([128, HB], dt, tag="x")
                            for q2 in range(2):
                                qg = h * 2 + q2
                                qs = slice(qg * BQ, (qg + 1) * BQ)
                                nc.tensor.matmul(x_h[:, q2 * BQ:(q2 + 1) * BQ],
                                                 djx_t, zst[:, qs],
                                                 start=True, stop=True)
                            for k in range(K):
                                col = k * MT + j
                                nc.scalar.activation(
                                    slabs[k // 2][:, (k % 2) * B + h * HB:
                                                  (k % 2) * B + (h + 1) * HB],
                                    x_h[:, :], AF.Exp,
                                    bias=pv[:, col:col + 1],
                                    scale=pu[:, col:col + 1])
                            nc.scalar.activation(xx[:, hs], x_h[:, :],
                                                 AF.Square,
                                                 scale=psq[:, j:j + 1])
                            for q2 in range(2):
                                qg = h * 2 + q2
                                qs = slice(qg * BQ, (qg + 1) * BQ)
                                yq = qpool.tile([128, BQ], dt, tag="yq")
                                nc.tensor.matmul(yq[:, :], djy_t, zst[:, qs],
                                                 start=True, stop=True)
                                nc.scalar.activation(yy[:, qs], yq[:, :],
                                                     AF.Square,
                                                     scale=psq[:, j:j + 1])
                                f3q = qpool.tile([128, BQ], dt, tag="f3q")
                                nc.tensor.matmul(f3q[:, :], f3z_t, zst[:, qs],
                                                 start=True, stop=False)
                                nc.tensor.matmul(f3q[:, :], f3d_t, dst[:, qs],
                                                 start=False, stop=False)
                                nc.tensor.matmul(f3q[:, :], f3c_t, rhsc[:, qs],
                                                 start=False, stop=True)
                                nc.vector.tensor_add(xx[:, qs], xx[:, qs],
                                                     yy[:, qs])
                                nc.vector.tensor_scalar(xx[:, qs], xx[:, qs],
                                                        psgn[:, j:j + 1],
                                                        None, AO.mult)
                                nc.vector.tensor_add(xx[:, qs], xx[:, qs],
                                                     f3q[:, :])

                        nc.scalar.activation(EB[:, :], xx[:, :], AF.Exp)

                        sa = wpool.tile([128, B], dt, tag="sa")
                        sb = wpool.tile([128, B], dt, tag="sb")
                        sc = wpool.tile([128, B], dt, tag="sc")
                        nc.vector.tensor_add(sa[:, :], slabs[0][:, 0:B],
                                             slabs[0][:, B:2 * B])
                        nc.vector.tensor_add(sb[:, :], slabs[1][:, 0:B],
                                             slabs[1][:, B:2 * B])
                        nc.vector.tensor_add(sa[:, :], sa[:, :], sb[:, :])
                        nc.vector.tensor_add(sb[:, :], slabs[2][:, 0:B],
                                             slabs[2][:, B:2 * B])
                        nc.vector.tensor_add(sc[:, :], slabs[3][:, 0:B],
                                             slabs[3][:, B:2 * B])
                        nc.vector.tensor_add(sb[:, :], sb[:, :], sc[:, :])
                        nc.vector.tensor_add(sa[:, :], sa[:, :], sb[:, :])
                        nc.vector.tensor_mul(wgt[:, :], EB[:, :], sa[:, :])
                        wgts.append(wgt)

                with tc.tile_pool(name="tps", bufs=1, space="PSUM") as tpool:
                    tp = tpool.tile([128, B], dt, tag="tp")
                    for j in range(MT):
                        that_j = that_all[:, j * 128:(j + 1) * 128]
                        for q in range(NQ):
                            qs = slice(q * BQ, (q + 1) * BQ)
                            nc.tensor.matmul(tp[:, qs], that_j,
                                             wgts[j][:, qs],
                                             start=(j == 0),
                                             stop=(j == MT - 1))
                    ocp = wpool.tile([128, B], dt, tag="xx")
                    nc.vector.tensor_copy(ocp[:, :], tp[:, :])
                    nc.sync.dma_start(tout[:, :], ocp[:, :])

    nc.compile()
    return nc


def kernel(z_re, z_im, d_re, d_im, zj_re, zj_im, dj_re, dj_im,
           That_re, That_im, alpha, sig_par, sig_perp, _emulate=False):
    p = _prep(z_re, z_im, d_re, d_im, zj_re, zj_im, dj_re, dj_im,
              That_re, That_im, alpha, sig_par, sig_perp)
    maps = [_core_slices(p, c) for c in range(NCORES)]

    if _emulate:
        outs = [_emulate_core(m) for m in maps]
    else:
        from concourse.bass_utils import run_bass_kernel_spmd
        if "nc" not in _CACHE:
            _CACHE["nc"] = _build_bass()
        dev_maps = _device_maps(maps)
        res = run_bass_kernel_spmd(_CACHE["nc"], dev_maps,
                                   core_ids=list(range(NCORES)))
        outs = [res.results[c]["tout"] for c in range(NCORES)]

    full = np.zeros((128, B), np.float64)
    for o in outs:
        full += o.astype(np.float64)
    full = full.astype(f32).T                   # [B, 128]
    return (full[:, :S] + 1j * full[:, S:]).astype(np.complex64)
